# revision 1
# baseline (speedup 1.0000x reference)
"""Multi-head attention forward on 8 Trainium2 NeuronCores.

Problem (all shapes hardcoded): B=2, S=2048, D=1024, H=16, HD=64
    q = relu(x @ Wq + bq); k = relu(x @ Wk + bk); v = relu(x @ Wv + bv)
    attn = softmax(q k^T / sqrt(HD)) per (batch, head)
    out = relu((attn @ v) @ Wo + bo)

Sharding: head-parallel for QKV+attention (2 heads per core, both batches),
then an AllToAll re-shards the per-head context to a per-token shard and each
core runs the full output projection for its 512 tokens. Host concatenates.

Device algorithm (per core):
  phase 1: Q^T, K^T ([64, 4096] per head, d on partitions) and V in natural
           layout augmented with a ones-column ([128 tokens, 65] per head
           block) -- all via bf16 matmuls against x^T with fp32 PSUM accum.
  phase 2: scores computed transposed, S^T[k, q] = K^T.T @ Q^T per head
           (K_c=64); exp on ACT straight out of PSUM with the 1/8 scale
           folded in (no max-subtraction: scores are O(1), fp32-exp safe);
           ctx^T = V_aug.T @ P accumulated over key blocks -- row 64 of the
           M=65 result is the softmax denominator for free.  Normalize with
           DVE reciprocal + GPSIMD partition-broadcast.
  phase 3: AllToAll of ctx^T token chunks; out = relu(ctx^T.T @ Wo + bo) for
           this core's 512 tokens.
"""

import os
import sys

import numpy as np

for _p in ("/opt/trn_rl_repo",):
    if os.path.isdir(_p) and _p not in sys.path:
        sys.path.append(_p)

import ml_dtypes

B, S, D, H = 2, 2048, 1024, 16
HD = D // H          # 64
NCORES = 8
T = B * S            # 4096 flattened tokens
DC = D // NCORES     # 128 head-dim columns per core (2 heads)
TCH = T // NCORES    # 512 tokens per core after the AllToAll
P = 128
KT_TILES = D // P    # 8 contraction tiles over d_model
NQC = T // 512       # 8 projection chunks of 512 tokens
NTB = T // P         # 32 token blocks of 128
SB_Q = S // 512      # 4 query chunks per batch
KB = S // P          # 16 key blocks per batch

_bf = ml_dtypes.bfloat16

# set by test.py for profiling; harness default is a plain run
PROFILE = False
LAST_RESULTS = None

_CACHE = {}


def _build(with_bias_v, with_bias_o, with_bias_qk):
    import concourse.mybir as mybir
    import concourse.tile as tile
    from concourse import bacc
    from concourse.bass import ds, ts
    from contextlib import ExitStack

    f32 = mybir.dt.float32
    bf16 = mybir.dt.bfloat16
    DT = bf16
    AF = mybir.ActivationFunctionType

    nc = bacc.Bacc("TRN2", target_bir_lowering=False, debug=False,
                   num_devices=NCORES)

    xT = nc.dram_tensor("xT", [D, T], DT, kind="ExternalInput")
    wq = nc.dram_tensor("wq", [D, DC], DT, kind="ExternalInput")
    wk = nc.dram_tensor("wk", [D, DC], DT, kind="ExternalInput")
    wv = nc.dram_tensor("wv", [D, DC], DT, kind="ExternalInput")
    wo = nc.dram_tensor("wo", [D, D], DT, kind="ExternalInput")
    bqd = nc.dram_tensor("bqv", [DC, 1], f32, kind="ExternalInput")
    bkd = nc.dram_tensor("bkv", [DC, 1], f32, kind="ExternalInput")
    bvd = nc.dram_tensor("bvv", [1, DC], DT, kind="ExternalInput")
    bod = nc.dram_tensor("bov", [1, D], DT, kind="ExternalInput")
    out = nc.dram_tensor("out", [TCH, D], f32, kind="ExternalOutput")

    with tile.TileContext(nc) as tc, ExitStack() as ctx:
        sb = ctx.enter_context(tc.tile_pool(name="persist", bufs=1))
        dram = ctx.enter_context(tc.tile_pool(name="dram", bufs=1, space="DRAM"))

        # persistent SBUF tensors
        xts = sb.tile([P, KT_TILES, T], DT)                 # x^T, 64KB/part
        qt = [sb.tile([HD, T], DT, name=f"qt{h}") for h in range(2)]
        kt = [sb.tile([HD, T], DT, name=f"kt{h}") for h in range(2)]
        va = sb.tile([P, NTB, 2, HD + 1], DT)               # V_aug
        wq_s = sb.tile([P, KT_TILES, DC], DT)
        wk_s = sb.tile([P, KT_TILES, DC], DT)
        wv_s = sb.tile([P, KT_TILES, DC], DT)
        wo_s = sb.tile([P, KT_TILES, D], DT)
        ctxt = sb.tile([P, KT_TILES, TCH], DT)              # gathered ctx^T
        ones = sb.tile([1, P], DT)
        bq_s = sb.tile([DC, 1], f32)
        bk_s = sb.tile([DC, 1], f32)
        bv_s = sb.tile([1, DC], DT)
        bo_s = sb.tile([1, D], DT)

        nc.vector.memset(ones[:], 1.0)
        nc.vector.memset(va[:], 1.0)  # ones column at [..., 64] survives
        if with_bias_qk:
            nc.sync.dma_start(out=bq_s[:], in_=bqd.ap())
            nc.sync.dma_start(out=bk_s[:], in_=bkd.ap())
        if with_bias_v:
            nc.sync.dma_start(out=bv_s[:], in_=bvd.ap())
        if with_bias_o:
            nc.sync.dma_start(out=bo_s[:], in_=bod.ap())

        # weight loads
        nc.sync.dma_start(out=wq_s[:], in_=wq.ap().rearrange("(k p) c -> p k c", p=P))
        nc.sync.dma_start(out=wk_s[:], in_=wk.ap().rearrange("(k p) c -> p k c", p=P))
        nc.sync.dma_start(out=wv_s[:], in_=wv.ap().rearrange("(k p) c -> p k c", p=P))
        wo3 = wo.ap().rearrange("(k p) e -> k p e", p=P)
        for kti in range(KT_TILES):
            nc.sync.dma_start(out=wo_s[:, kti], in_=wo3[kti])

        # x^T streamed in 512-token chunks so the first projection chunk's
        # inputs arrive quickly
        xT3 = xT.ap().rearrange("(k p) t -> k p t", p=P)
        for qc in range(NQC):
            for kti in range(KT_TILES):
                nc.sync.dma_start(out=xts[:, kti, ts(qc, 512)],
                                  in_=xT3[kti][:, ts(qc, 512)])

        # ---------------- phase 1: projections ----------------
        with tc.tile_pool(name="p1ps", bufs=3, space="PSUM") as p1ps:
            for qc in range(NQC):
                for (w_s, b_s, dsts, wb) in (
                    (wq_s, bq_s, qt, with_bias_qk),
                    (wk_s, bk_s, kt, with_bias_qk),
                ):
                    ps = p1ps.tile([P, 512], mybir.dt.float32, tag="qk")
                    for kti in range(KT_TILES):
                        nc.tensor.matmul(ps[:], w_s[:, kti], xts[:, kti, ts(qc, 512)],
                                         start=(kti == 0), stop=(kti == KT_TILES - 1))
                    for h in range(2):
                        sl = ps[h * HD:(h + 1) * HD, :]
                        if wb:
                            nc.scalar.activation(dsts[h][:, ts(qc, 512)], sl,
                                                 AF.Relu, bias=b_s[h * HD:(h + 1) * HD, :])
                        else:
                            nc.vector.tensor_scalar_max(dsts[h][:, ts(qc, 512)], sl, 0.0)
                # V natural layout for this chunk's 4 token blocks
                for tb in range(4 * qc, 4 * qc + 4):
                    vps = p1ps.tile([P, DC], mybir.dt.float32, tag="v")
                    if with_bias_v:
                        nc.tensor.matmul(vps[:], ones[:], bv_s[:], start=True, stop=False)
                    for kti in range(KT_TILES):
                        nc.tensor.matmul(vps[:], xts[:, kti, ts(tb, P)], wv_s[:, kti],
                                         start=(kti == 0 and not with_bias_v),
                                         stop=(kti == KT_TILES - 1))
                    for h in range(2):
                        nc.vector.tensor_scalar_max(va[:, tb, h, 0:HD],
                                                    vps[:, h * HD:(h + 1) * HD], 0.0)

        # ---------------- phase 2: attention ----------------
        a2a_in = dram.tile([NCORES, P, TCH], DT)
        a2a_out = dram.tile([NCORES, P, TCH], DT)

        with tc.tile_pool(name="scps", bufs=2, space="PSUM") as scps, \
             tc.tile_pool(name="ctxps", bufs=4, space="PSUM") as ctxps, \
             tc.tile_pool(name="ptp", bufs=3) as ptp, \
             tc.tile_pool(name="nrm", bufs=4) as nrm:
            for b in range(B):
                for qc in range(SB_Q):
                    j = b * SB_Q + qc           # global token chunk == A2A slot
                    qsl = ds(b * S + qc * 512, 512)
                    cps = [ctxps.tile([HD + 1, 512], mybir.dt.float32,
                                      tag="ctx", name=f"cps{h}") for h in range(2)]
                    for kb in range(KB):
                        ksl = ds(b * S + kb * P, P)
                        sps = scps.tile([P, 2, 512], mybir.dt.float32, tag="sc")
                        for h in range(2):
                            nc.tensor.matmul(sps[:, h], kt[h][:, ksl], qt[h][:, qsl],
                                             start=True, stop=True)
                        pt = ptp.tile([P, 2, 512], DT, tag="p")
                        nc.scalar.activation(pt[:], sps[:], AF.Exp, scale=0.125)
                        for h in range(2):
                            nc.tensor.matmul(cps[h][:], va[:, b * KB + kb, h], pt[:, h],
                                             start=(kb == 0), stop=(kb == KB - 1))
                    for h in range(2):
                        rec = nrm.tile([1, 512], mybir.dt.float32, tag="rec")
                        nc.vector.reciprocal(rec[:], cps[h][HD:HD + 1, :])
                        recb = nrm.tile([HD, 512], mybir.dt.float32, tag="recb")
                        nc.gpsimd.partition_broadcast(recb[:], rec[0:1, :])
                        csb = nrm.tile([HD, 512], DT, tag="csb")
                        nc.vector.tensor_tensor(csb[:], cps[h][0:HD, :], recb[:],
                                                mybir.AluOpType.mult)
                        nc.sync.dma_start(out=a2a_in[j, h * HD:(h + 1) * HD, :],
                                          in_=csb[:])

            nc.gpsimd.collective_compute(
                "AllToAll", mybir.AluOpType.bypass,
                replica_groups=[list(range(NCORES))],
                ins=[a2a_in.opt()], outs=[a2a_out.opt()],
            )

        # ---------------- phase 3: output projection ----------------
        with tc.tile_pool(name="p3ps", bufs=4, space="PSUM") as p3ps, \
             tc.tile_pool(name="p3sb", bufs=3) as p3sb:
            for i in range(NCORES):
                nc.sync.dma_start(out=ctxt[:, i], in_=a2a_out[i])
            for tb in range(TCH // P):
                for ec in range(D // 512):
                    ps = p3ps.tile([P, 512], mybir.dt.float32, tag="o")
                    if with_bias_o:
                        nc.tensor.matmul(ps[:], ones[:], bo_s[:, ts(ec, 512)],
                                         start=True, stop=False)
                    for kti in range(KT_TILES):
                        nc.tensor.matmul(ps[:], ctxt[:, kti, ts(tb, P)],
                                         wo_s[:, kti, ts(ec, 512)],
                                         start=(kti == 0 and not with_bias_o),
                                         stop=(kti == KT_TILES - 1))
                    osb = p3sb.tile([P, 512], mybir.dt.float32, tag="osb")
                    nc.vector.tensor_scalar_max(osb[:], ps[:], 0.0)
                    nc.sync.dma_start(out=out.ap()[ts(tb, P), ts(ec, 512)], in_=osb[:])

    nc.compile()
    return nc


def _get(with_bias_v, with_bias_o, with_bias_qk):
    key = (with_bias_v, with_bias_o, with_bias_qk)
    if key not in _CACHE:
        _CACHE[key] = _build(*key)
    return _CACHE[key]


def kernel(x, Wq, bq, Wk, bk, Wv, bv, Wo, bo):
    global LAST_RESULTS
    from concourse.bass_utils import run_bass_kernel_spmd

    x = np.asarray(x, dtype=np.float32)
    Wq, Wk, Wv, Wo = (np.asarray(w, dtype=np.float32) for w in (Wq, Wk, Wv, Wo))
    bq, bk, bv, bo = (np.asarray(v, dtype=np.float32) for v in (bq, bk, bv, bo))

    wb_qk = bool(np.any(bq) or np.any(bk))
    wb_v = bool(np.any(bv))
    wb_o = bool(np.any(bo))
    nc = _get(wb_v, wb_o, wb_qk)

    xT = np.ascontiguousarray(x.reshape(T, D).astype(_bf).T)
    Wq16 = Wq.astype(_bf)
    Wk16 = Wk.astype(_bf)
    Wv16 = Wv.astype(_bf)
    Wo16 = np.ascontiguousarray(Wo.astype(_bf))
    bv16 = bv.astype(_bf)
    bo16 = np.ascontiguousarray(bo.astype(_bf).reshape(1, D))

    in_maps = []
    for c in range(NCORES):
        cs = slice(c * DC, (c + 1) * DC)
        in_maps.append({
            "xT": xT,
            "wq": np.ascontiguousarray(Wq16[:, cs]),
            "wk": np.ascontiguousarray(Wk16[:, cs]),
            "wv": np.ascontiguousarray(Wv16[:, cs]),
            "wo": Wo16,
            "bqv": np.ascontiguousarray(bq[cs].reshape(DC, 1)),
            "bkv": np.ascontiguousarray(bk[cs].reshape(DC, 1)),
            "bvv": np.ascontiguousarray(bv16[cs].reshape(1, DC)),
            "bov": bo16,
        })

    kw = {}
    if PROFILE:
        kw = dict(trace=True, trace_cores=[0])
    res = run_bass_kernel_spmd(nc, in_maps, core_ids=list(range(NCORES)), **kw)
    LAST_RESULTS = res
    full = np.concatenate([res.results[c]["out"] for c in range(NCORES)], axis=0)
    return np.ascontiguousarray(full.reshape(B, S, D).astype(np.float32))
